# revision 37
# baseline (speedup 1.0000x reference)
"""LLaMA causal self-attention, 8-way head-tensor-parallel Trainium2 Bass kernel.

Sharding: each of 8 cores computes 4 query heads + its 1 KV head-group
(Wq/Wk/Wv column-sharded), plus a row-shard of Wo producing a partial
(S, DIM) output; partials are summed on the host (the all-reduce of the
row-sharded Wo matmul).

v2 layout/perf notes:
  - Everything on-device is bf16 (PSUM accumulation stays fp32): halves HBM
    traffic for x / weights / partial output and doubles DVE throughput.
  - Setup weight loads are spread across the two HWDGE queues (sync, scalar)
    so the first projection matmul starts ~3us in instead of ~30us.
  - q/k channel order is host-permuted per head to [evens, odds]; RoPE pair
    mixing uses 32-row SBUF->SBUF swap DMAs on the scalar HWDGE queue.
  - scores are computed transposed (scoresT[k, q]): per (j, head-pair) the two
    K=64 score matmuls write adjacent PSUM banks of one [128,2,512] tile with
    alternating row-groups (0,0)/(64,0) (concurrent on HW), and a single
    1024-wide exp activation covers both heads (amortizes ACT overhead).
  - v is transposed via the 2-byte DMA-transpose XBAR (no PE/psum involved);
    a ones-column appended to v yields softmax row-sums as psum row 64.
  - softmax normalization: DVE reciprocal straight off the psum row, rank-1
    (K=1) PE matmul broadcast, one DVE multiply.
"""

import numpy as np
import ml_dtypes  # noqa: F401  (registers bfloat16 numpy dtype)

import concourse.bass as bass
import concourse.mybir as mybir
import concourse.tile as tile
from contextlib import nullcontext
from concourse import bacc
from concourse.bass import ts, ds
from concourse.bass_utils import run_bass_kernel_spmd
from concourse.masks import make_identity

F32 = mybir.dt.float32
F32R = mybir.dt.float32r
BF16 = mybir.dt.bfloat16

S = 2048
DIM = 2048
H = 32
KVH = 8
D = 64
NCORES = 8
HQ = H // NCORES          # 4 q heads per core
CQ = HQ * D               # 256 q cols per core
ST = 512                  # s-tile width in QKV projection phase
QT = 512                  # q-tile width in attention
NKT = S // 128            # 16 key tiles
NDT = DIM // 128          # 16 contraction tiles for projections
NST = S // ST             # 4 projection s-tiles
NQT = S // QT             # 4 attention q-tiles

USE_DMA_TRANSPOSE = False
DEBUG_DUMP = False


def _build(causal: bool, use_mask: bool):
    nc = bacc.Bacc("TRN2", target_bir_lowering=False, debug=False,
                   num_devices=NCORES, name="llama_attn")
    xT = nc.dram_tensor("xT", [DIM, S], BF16, kind="ExternalInput")
    wq = nc.dram_tensor("wq", [DIM, CQ], BF16, kind="ExternalInput")
    wkv = nc.dram_tensor("wkv", [DIM, 128], BF16, kind="ExternalInput")
    wo = nc.dram_tensor("wo", [CQ, DIM], BF16, kind="ExternalInput")
    bqd = nc.dram_tensor("bq", [CQ], F32, kind="ExternalInput")
    bkvd = nc.dram_tensor("bkv", [128], F32, kind="ExternalInput")
    ccd = nc.dram_tensor("cc", [128, S], BF16, kind="ExternalInput")
    ssd = nc.dram_tensor("ssgn", [128, S], BF16, kind="ExternalInput")
    onesd = nc.dram_tensor("ones", [128], BF16, kind="ExternalInput")
    onesfd = nc.dram_tensor("onesf", [64], F32R, kind="ExternalInput")
    if causal:
        trid = nc.dram_tensor("trimask", [128, 4, 2, QT], BF16,
                              kind="ExternalInput")
    if use_mask:
        maskt = nc.dram_tensor("maskt", [S, S], BF16, kind="ExternalInput")
    partial = nc.dram_tensor("partial", [S, DIM], BF16, kind="ExternalOutput")
    if DEBUG_DUMP:
        kT_d = nc.dram_tensor("kT_d", [128, S], BF16, kind="ExternalOutput")
        qT_d = nc.dram_tensor("qT_d", [128, S], BF16, kind="ExternalOutput")
        v_d = nc.dram_tensor("v_d", [128, NKT * 65], BF16, kind="ExternalOutput")
        attn_d = nc.dram_tensor("attn_d", [128, QT], BF16, kind="ExternalOutput")
        pt_d = nc.dram_tensor("pt_d", [128, 2, 2, QT], BF16, kind="ExternalOutput")
        aps_d = nc.dram_tensor("aps_d", [65, 4, QT], F32, kind="ExternalOutput")
        bcs_d = nc.dram_tensor("bcs_d", [64, 4, QT], F32, kind="ExternalOutput")
        rc_d = nc.dram_tensor("rc_d", [65, 4, QT], F32, kind="ExternalOutput")

    with tile.TileContext(nc) as tc:
        with tc.tile_pool(name="persist", bufs=1) as pp:
            # --- setup loads, spread across the two HWDGE queues.
            # scalar queue: wq in k-tile chunks (first projection matmul can
            # start after chunk 0), wkv, biases. sync queue: x stream (phase 1
            # loop below), rope tables, then wo/tri (needed only in phase 2).
            wq_sb = pp.tile([128, NDT, CQ], BF16)
            for wc in range(4):
                nc.scalar.dma_start(wq_sb[:, ts(wc, 4), :],
                                    wq.ap().rearrange("(a p) c -> p a c", p=128)
                                    [:, ts(wc, 4), :])
            wkv_sb = pp.tile([128, NDT, 128], BF16)
            nc.scalar.dma_start(wkv_sb[:], wkv.ap().rearrange("(a p) c -> p a c", p=128))
            bq_sb = pp.tile([128, 2], F32)
            nc.scalar.dma_start(bq_sb[:], bqd.ap().rearrange("(t p) -> p t", p=128))
            bkv_sb = pp.tile([128, 1], F32)
            nc.scalar.dma_start(bkv_sb[:], bkvd.ap()[:, None])
            ones_pe = pp.tile([65, 64], F32R, name="ones_pe")
            nc.gpsimd.dma_start(ones_pe[64:65, :], onesfd.ap()[None, :])
            cc_sb = pp.tile([128, S], BF16)
            ss_sb = pp.tile([128, S], BF16)
            wo_sb = pp.tile([128, 2, DIM], BF16)
            if causal:
                tri_sb = pp.tile([128, 4, 2, QT], BF16)
            if not USE_DMA_TRANSPOSE:
                ident = pp.tile([64, 64], BF16)
                make_identity(nc, ident[:])

            # persistent activations
            qT = [pp.tile([128, S], BF16, tag=f"qt{i}", name=f"qt{i}") for i in range(2)]
            kT = pp.tile([128, S], BF16)          # rows 0:64 and 64:128 both = roped k
            v_sb = pp.tile([128, NKT, 65], BF16)  # [kpos, ktile, ch + ones]
            nc.gpsimd.dma_start(v_sb[:, :, 64:65],
                                onesd.ap()[:, None, None].to_broadcast((128, NKT, 1)))
            attn = [[pp.tile([128, QT], BF16, tag=f"attn{c}_{t}", name=f"attn{c}_{t}")
                     for t in range(NQT)] for c in range(2)]

            # ------- Phase 1: QKV projections + fused RoPE + v transpose -------
            with tc.tile_pool(name="xstream", bufs=2) as xp, \
                 tc.tile_pool(name="qkvps", bufs=2, space="PSUM") as qps, \
                 (nullcontext() if USE_DMA_TRANSPOSE else
                  tc.tile_pool(name="vtps", bufs=2, space="PSUM")) as vps, \
                 tc.tile_pool(name="rawp", bufs=2) as rawp, \
                 tc.tile_pool(name="rope", bufs=3) as rp:

                def rope(raw_ap, out_ap, rows, ssl):
                    sw = rp.tile([128, ST], BF16, tag="ropesw")
                    t1 = rp.tile([128, ST], BF16, tag="ropet1")
                    for b in rows:
                        nc.gpsimd.dma_start(sw[b:b + 32, :], raw_ap[b + 32:b + 64, :])
                        nc.gpsimd.dma_start(sw[b + 32:b + 64, :], raw_ap[b:b + 32, :])
                    lo, hi = rows[0], rows[-1] + 64
                    nc.vector.tensor_mul(t1[lo:hi, :], raw_ap[lo:hi, :], cc_sb[lo:hi, ssl])
                    nc.vector.tensor_mul(sw[lo:hi, :], sw[lo:hi, :], ss_sb[lo:hi, ssl])
                    nc.vector.tensor_add(out_ap[lo:hi, ssl], t1[lo:hi, :], sw[lo:hi, :])

                for st in range(NST):
                    ssl = ts(st, ST)
                    xt = xp.tile([128, NDT, ST], BF16, tag="xt")
                    xr = xT.ap().rearrange("(a p) s -> p a s", p=128)
                    for qtr in range(4):
                        nc.sync.dma_start(xt[:, ts(qtr, 4), :], xr[:, ts(qtr, 4), ssl])
                    if st == 0:
                        # rope tables after the first x tile; phase-2-only
                        # tensors at the back of the sync queue
                        nc.sync.dma_start(cc_sb[:], ccd[:])
                        nc.sync.dma_start(ss_sb[:], ssd[:])
                    if st == NST - 1:
                        nc.sync.dma_start(wo_sb[:],
                                          wo.ap().rearrange("(a p) e -> p a e", p=128))
                        if causal:
                            nc.sync.dma_start(tri_sb[:], trid[:])
                    pq0 = qps.tile([128, ST], F32, tag="q0")
                    pq1 = qps.tile([128, ST], F32, tag="q1")
                    pkv = qps.tile([128, ST], F32, tag="kv")
                    for kt in range(NDT):
                        st_flag, sp_flag = kt == 0, kt == NDT - 1
                        nc.tensor.matmul(pq0[:], wq_sb[:, kt, 0:128], xt[:, kt, :],
                                         start=st_flag, stop=sp_flag)
                        nc.tensor.matmul(pq1[:], wq_sb[:, kt, 128:256], xt[:, kt, :],
                                         start=st_flag, stop=sp_flag)
                        nc.tensor.matmul(pkv[:], wkv_sb[:, kt, :], xt[:, kt, :],
                                         start=st_flag, stop=sp_flag)
                    q0_raw = rawp.tile([128, ST], BF16, tag="q0r")
                    q1_raw = rawp.tile([128, ST], BF16, tag="q1r")
                    kv_raw = rawp.tile([128, ST], BF16, tag="kvr")
                    nc.scalar.activation(q0_raw[:], pq0[:],
                                         mybir.ActivationFunctionType.Identity,
                                         bias=bq_sb[:, 0:1])
                    nc.scalar.activation(q1_raw[:], pq1[:],
                                         mybir.ActivationFunctionType.Identity,
                                         bias=bq_sb[:, 1:2])
                    nc.scalar.activation(kv_raw[:], pkv[:],
                                         mybir.ActivationFunctionType.Identity,
                                         bias=bkv_sb[:, 0:1])
                    rope(q0_raw[:], qT[0][:], [0, 64], ssl)
                    rope(q1_raw[:], qT[1][:], [0, 64], ssl)
                    rope(kv_raw[:], kT[:], [64], ssl)
                    nc.gpsimd.dma_start(kT[0:64, ssl], kT[64:128, ssl])
                    for vc in range(ST // 128):
                        j = (st * ST) // 128 + vc
                        if USE_DMA_TRANSPOSE:
                            nc.sync.dma_start(v_sb[:, j, 0:64],
                                              kv_raw[0:64, ts(vc, 128)],
                                              transpose=True)
                        else:
                            vt_ps = vps.tile([128, 64], BF16, tag="vt")
                            nc.tensor.transpose(vt_ps[:], kv_raw[0:64, ts(vc, 128)],
                                                ident[:])
                            nc.vector.tensor_copy(v_sb[:, j, 0:64], vt_ps[:])

            # ------- Phase 2+3 fused: attention with interleaved out-proj -------
            with tc.tile_pool(name="scps", bufs=2, space="PSUM") as scps, \
                 tc.tile_pool(name="avps", bufs=4, space="PSUM") as avps, \
                 tc.tile_pool(name="ptp", bufs=16) as ptp, \
                 tc.tile_pool(name="nrm", bufs=3) as nrm, \
                 tc.tile_pool(name="osb", bufs=8) as osb, \
                 (tc.tile_pool(name="mskp", bufs=4) if use_mask else nullcontext()) as mskp:
                def emit_outproj(pps, t, sl, et, eng="dve"):
                    for ct in range(2):
                        nc.tensor.matmul(pps[:],
                                         attn[ct][t][:, ts(sl, 128)],
                                         wo_sb[:, ct, ts(et, 512)],
                                         start=(ct == 0), stop=(ct == 1))
                    ot = osb.tile([128, QT], BF16, tag="ot")
                    if eng == "act":
                        nc.scalar.activation(ot[:], pps[:],
                                             mybir.ActivationFunctionType.Identity)
                    else:
                        nc.vector.tensor_copy(ot[:], pps[:])
                    nc.sync.dma_start(
                        partial[ts(4 * t + sl, 128), ts(et, 512)], ot[:])

                def finish_norm(t, svs, rcs):
                    # bc matmuls + normalization multiplies for t, emitted
                    # early in t+1's j-loop so the in-order PE queue reaches
                    # them only once their reciprocals are long done
                    for h in range(4):
                        hp, hh = divmod(h, 2)
                        bc = scps.tile([64, QT], F32, tag="sc", name="bc")
                        nc.tensor.matmul(bc[:], ones_pe[64:65, 0:64],
                                         rcs[h][64:65, :],
                                         start=True, stop=True)
                        if DEBUG_DUMP and t == 0:
                            bcs = nrm.tile([64, QT], F32, tag="bcs")
                            nc.vector.tensor_copy(bcs[:], bc[:])
                            nc.sync.dma_start(bcs_d.ap()[:, h], bcs[:])
                        if hh == 0:
                            nc.vector.tensor_mul(attn[hp][t][0:64, :],
                                                 svs[h][0:64, :], bc[:])
                        else:
                            tb = nrm.tile([64, QT], BF16, tag="tb")
                            nc.vector.tensor_mul(tb[:], svs[h][0:64, :], bc[:])
                            nc.gpsimd.dma_start(attn[hp][t][64:128, :], tb[:])

                pending = []   # out-proj work of the previous t, interleaved
                norm_prev = None  # (t-1, aps, reciprocal rows)
                for t in range(NQT):
                    tsl = ts(t, QT)
                    n_k = 4 * (t + 1) if causal else NKT
                    # slot rotation: aps(t) land on the slots of op(t-2)#12..15
                    # (long done), and op(t-1) tiles land on aps(t-1)'s slots,
                    # which free exactly when t-1's norm multiplies complete
                    aps = [avps.tile([65, QT], F32, tag="av", name=f"av{t}_{h}")
                           for h in range(4)]
                    pending = [(avps.tile([128, QT], F32, tag="av", name="op"),
                                *w) for w in pending]
                    for j in range(n_k):
                        jsl = ts(j, 128)
                        if j == 1 and norm_prev is not None:
                            finish_norm(*norm_prev)
                            norm_prev = None
                        if j >= 2:
                            for _ in range(2):
                                if pending:
                                    emit_outproj(*pending.pop(0))
                        if use_mask:
                            mt = mskp.tile([128, QT], BF16, tag="mt")
                            nc.sync.dma_start(mt[:], maskt[jsl, tsl])
                        scs = []
                        for hp in range(2):
                            sc = scps.tile([128, 2, QT], F32, tag="sc", name="sc")
                            nc.tensor.matmul(sc[:, 0, :], kT[0:64, jsl],
                                             qT[hp][0:64, tsl],
                                             start=True, stop=True)
                            scs.append(sc)
                        for hp in range(2):
                            nc.tensor.matmul(scs[hp][:, 1, :], kT[64:128, jsl],
                                             qT[hp][64:128, tsl],
                                             start=True, stop=True)
                        pts_dbg = []
                        for hp in range(2):
                            pt = ptp.tile([128, 2, QT], BF16, tag="pt")
                            pts_dbg.append(pt)
                            nc.scalar.activation(pt[:], scs[hp][:],
                                                 mybir.ActivationFunctionType.Exp,
                                                 scale=0.125)
                            if causal and j >= 4 * t:
                                nc.vector.tensor_mul(pt[:], pt[:],
                                                     tri_sb[:, j - 4 * t])
                            if use_mask:
                                for c in range(2):
                                    nc.vector.tensor_mul(pt[:, c, :], pt[:, c, :],
                                                         mt[:])
                            for c in range(2):
                                nc.tensor.matmul(aps[2 * hp + c][:], v_sb[:, j, :],
                                                 pt[:, c, :],
                                                 start=(j == 0), stop=(j == n_k - 1))
                        if DEBUG_DUMP and t == 0 and j == 0:
                            for hpd in range(2):
                                nc.sync.dma_start(pt_d.ap()[:, hpd], pts_dbg[hpd][:])
                    # drain any out-proj leftovers of t-1 (short j-loops)
                    while pending:
                        emit_outproj(*pending.pop(0))
                    # reciprocals of this t's softmax sums (DVE only; the
                    # dependent bc matmuls and multiplies are deferred)
                    rcs, svs = [], []
                    for h in range(4):
                        sv = nrm.tile([65, QT], F32, tag="sv", bufs=8)
                        nc.scalar.activation(sv[:], aps[h][:],
                                             mybir.ActivationFunctionType.Identity)
                        if DEBUG_DUMP and t == 0:
                            nc.sync.dma_start(aps_d.ap()[:, h], sv[:])
                        svs.append(sv)
                        rc = nrm.tile([65, QT], F32, tag="rc", bufs=8)
                        nc.vector.reciprocal_approx_fast(rc[:], sv[:])
                        if DEBUG_DUMP and t == 0:
                            nc.sync.dma_start(rc_d.ap()[:, h], rc[:])
                        rcs.append(rc)
                    rcbs = []
                    for h in range(4):
                        rcb = nrm.tile([65, QT], F32R, tag="rcb", bufs=8)
                        nc.scalar.activation(rcb[64:65, :], rcs[h][64:65, :],
                                             mybir.ActivationFunctionType.Copy)
                        rcbs.append(rcb)
                    norm_prev = (t, svs, rcbs)
                    pending = [(t, sl, et) for sl in range(4) for et in range(4)]
                if DEBUG_DUMP:
                    nc.sync.dma_start(kT_d[:], kT[:])
                    nc.sync.dma_start(qT_d[:], qT[0][:])
                    nc.sync.dma_start(v_d.ap().rearrange("p (a c) -> p a c", a=NKT), v_sb[:])
                    nc.sync.dma_start(attn_d[:], attn[0][0][:])
                # tail: the last t's norm + out-projection
                finish_norm(*norm_prev)
                for i, w in enumerate(pending):
                    emit_outproj(avps.tile([128, QT], F32, tag="av", name="op"), *w,
                                 eng="act" if i % 2 else "dve")

    nc.compile()
    return nc


_CACHE = {}
TRACE = False
LAST_EXEC_NS = None
LAST_RES = None


def _get(causal, use_mask):
    key = (causal, use_mask)
    if key not in _CACHE:
        _CACHE[key] = _build(causal, use_mask)
    return _CACHE[key]


def _perm_eo(w):
    # de-interleave channel pairs per 64-col head block: [evens, odds]
    cols = np.concatenate([np.arange(0, 64, 2), np.arange(1, 64, 2)])
    return w[..., cols]


def _bf(a):
    return np.ascontiguousarray(np.asarray(a, dtype=np.float32).astype(ml_dtypes.bfloat16))


def kernel(**inputs):
    x = np.asarray(inputs["x"], dtype=np.float32)
    fc = np.asarray(inputs["freqs_cos"], dtype=np.float32)
    fs = np.asarray(inputs["freqs_sin"], dtype=np.float32)
    mask = np.asarray(inputs["mask"])
    Wq = np.asarray(inputs["Wq"], dtype=np.float32)
    bq = np.asarray(inputs["bq"], dtype=np.float32)
    Wk = np.asarray(inputs["Wk"], dtype=np.float32)
    bk = np.asarray(inputs["bk"], dtype=np.float32)
    Wv = np.asarray(inputs["Wv"], dtype=np.float32)
    bv = np.asarray(inputs["bv"], dtype=np.float32)
    Wo = np.asarray(inputs["Wo"], dtype=np.float32)
    bo = np.asarray(inputs["bo"], dtype=np.float32)

    m2 = mask.reshape(S, S)
    if (m2 == 1).all():
        causal, use_mask = False, False
    elif np.array_equal(m2 != 0, np.tril(np.ones((S, S), dtype=bool))):
        causal, use_mask = True, False
    else:
        causal, use_mask = False, True
    nc = _get(causal, use_mask)

    xT = _bf(x[0].T)
    cosT = np.asarray(fc.T, dtype=np.float32)  # (32, S)
    sinT = np.asarray(fs.T, dtype=np.float32)
    cc = _bf(np.tile(cosT, (4, 1)))
    ssgn = _bf(np.concatenate([-sinT, sinT, -sinT, sinT], axis=0))
    kl = np.arange(128)[:, None]
    qq = np.arange(QT)[None, :]
    tri = np.stack([(qq >= 128 * v + kl) for v in range(4)]).astype(np.float32)
    tri2 = _bf(np.broadcast_to(tri[None, :, :, :], (2, 4, 128, QT))
               .transpose(2, 1, 0, 3))  # [128, 4, 2, QT]

    Wq_h = Wq.reshape(DIM, H, D)
    bq_h = bq.reshape(H, D)
    Wk_h = Wk.reshape(DIM, KVH, D)
    bk_h = bk.reshape(KVH, D)

    in_maps = []
    for c in range(NCORES):
        hs = slice(HQ * c, HQ * (c + 1))
        wq_c = _perm_eo(Wq_h[:, hs, :]).reshape(DIM, CQ)
        bq_c = _perm_eo(bq_h[hs, :]).reshape(CQ)
        wk_c = _perm_eo(Wk_h[:, c, :])
        bk_c = _perm_eo(bk_h[c, :])
        wv_c = Wv[:, 64 * c:64 * (c + 1)]
        bv_c = bv[64 * c:64 * (c + 1)]
        wkv_c = np.concatenate([wv_c, wk_c], axis=1)
        bkv_c = np.concatenate([bv_c, bk_c])
        wo_c = Wo[CQ * c:CQ * (c + 1), :]
        im = {
            "xT": xT, "wq": _bf(wq_c), "wkv": _bf(wkv_c),
            "wo": _bf(wo_c), "bq": np.ascontiguousarray(bq_c),
            "bkv": np.ascontiguousarray(bkv_c), "cc": cc,
            "ssgn": ssgn,
            "ones": np.ones(128, dtype=ml_dtypes.bfloat16),
            "onesf": np.ones(64, dtype=np.float32),
        }
        if causal:
            im["trimask"] = tri2
        if use_mask:
            im["maskt"] = _bf(m2.T)
        in_maps.append(im)

    global LAST_EXEC_NS, LAST_RES
    res = run_bass_kernel_spmd(nc, in_maps, core_ids=list(range(NCORES)), trace=TRACE)
    LAST_EXEC_NS = res.exec_time_ns
    LAST_RES = res
    out = np.zeros((S, DIM), dtype=np.float32)
    for rr in res.results:
        out += np.asarray(rr["partial"], dtype=np.float32)
    out += bo
    return out.reshape(1, S, DIM)


# revision 40
# speedup vs baseline: 1.0713x; 1.0713x over previous
"""LLaMA causal self-attention, 8-way head-tensor-parallel Trainium2 Bass kernel.

Sharding: each of 8 cores computes 4 query heads + its 1 KV head-group
(Wq/Wk/Wv column-sharded), plus a row-shard of Wo producing a partial
(S, DIM) output; partials are summed on the host (the all-reduce of the
row-sharded Wo matmul).

v2 layout/perf notes:
  - Everything on-device is bf16 (PSUM accumulation stays fp32): halves HBM
    traffic for x / weights / partial output and doubles DVE throughput.
  - Setup weight loads are spread across the two HWDGE queues (sync, scalar)
    so the first projection matmul starts ~3us in instead of ~30us.
  - q/k channel order is host-permuted per head to [evens, odds]; RoPE pair
    mixing uses 32-row SBUF->SBUF swap DMAs on the scalar HWDGE queue.
  - scores are computed transposed (scoresT[k, q]): per (j, head-pair) the two
    K=64 score matmuls write adjacent PSUM banks of one [128,2,512] tile with
    alternating row-groups (0,0)/(64,0) (concurrent on HW), and a single
    1024-wide exp activation covers both heads (amortizes ACT overhead).
  - v is transposed via the 2-byte DMA-transpose XBAR (no PE/psum involved);
    a ones-column appended to v yields softmax row-sums as psum row 64.
  - softmax normalization: DVE reciprocal straight off the psum row, rank-1
    (K=1) PE matmul broadcast, one DVE multiply.
"""

import numpy as np
import ml_dtypes  # noqa: F401  (registers bfloat16 numpy dtype)

import concourse.bass as bass
import concourse.mybir as mybir
import concourse.tile as tile
from contextlib import nullcontext
from concourse import bacc
from concourse.bass import ts, ds
from concourse.bass_utils import run_bass_kernel_spmd
from concourse.masks import make_identity

F32 = mybir.dt.float32
F32R = mybir.dt.float32r
BF16 = mybir.dt.bfloat16

S = 2048
DIM = 2048
H = 32
KVH = 8
D = 64
NCORES = 8
HQ = H // NCORES          # 4 q heads per core
CQ = HQ * D               # 256 q cols per core
ST = 512                  # s-tile width in QKV projection phase
QT = 512                  # q-tile width in attention
NKT = S // 128            # 16 key tiles
NDT = DIM // 128          # 16 contraction tiles for projections
NST = S // ST             # 4 projection s-tiles
NQT = S // QT             # 4 attention q-tiles

USE_DMA_TRANSPOSE = False
DEBUG_DUMP = False


def _build(causal: bool, use_mask: bool):
    nc = bacc.Bacc("TRN2", target_bir_lowering=False, debug=False,
                   num_devices=NCORES, name="llama_attn")
    xT = nc.dram_tensor("xT", [DIM, S], BF16, kind="ExternalInput")
    wq = nc.dram_tensor("wq", [DIM, CQ], BF16, kind="ExternalInput")
    wkv = nc.dram_tensor("wkv", [DIM, 128], BF16, kind="ExternalInput")
    wo = nc.dram_tensor("wo", [CQ, DIM], BF16, kind="ExternalInput")
    bqd = nc.dram_tensor("bq", [CQ], F32, kind="ExternalInput")
    bkvd = nc.dram_tensor("bkv", [128], F32, kind="ExternalInput")
    ccd = nc.dram_tensor("cc", [128, S], BF16, kind="ExternalInput")
    ssd = nc.dram_tensor("ssgn", [128, S], BF16, kind="ExternalInput")
    onesd = nc.dram_tensor("ones", [128], BF16, kind="ExternalInput")
    onesfd = nc.dram_tensor("onesf", [64], F32R, kind="ExternalInput")
    if causal:
        trid = nc.dram_tensor("trimask", [128, 4, 2, QT], BF16,
                              kind="ExternalInput")
    if use_mask:
        maskt = nc.dram_tensor("maskt", [S, S], BF16, kind="ExternalInput")
    partial = nc.dram_tensor("partial", [S, DIM], BF16, kind="ExternalOutput")
    if DEBUG_DUMP:
        kT_d = nc.dram_tensor("kT_d", [128, S], BF16, kind="ExternalOutput")
        qT_d = nc.dram_tensor("qT_d", [128, S], BF16, kind="ExternalOutput")
        v_d = nc.dram_tensor("v_d", [128, NKT * 65], BF16, kind="ExternalOutput")
        attn_d = nc.dram_tensor("attn_d", [128, QT], BF16, kind="ExternalOutput")
        pt_d = nc.dram_tensor("pt_d", [128, 2, 2, QT], BF16, kind="ExternalOutput")
        aps_d = nc.dram_tensor("aps_d", [65, 4, QT], F32, kind="ExternalOutput")
        bcs_d = nc.dram_tensor("bcs_d", [64, 4, QT], F32, kind="ExternalOutput")
        rc_d = nc.dram_tensor("rc_d", [65, 4, QT], F32, kind="ExternalOutput")

    with tile.TileContext(nc) as tc:
        with tc.tile_pool(name="persist", bufs=1) as pp:
            # --- setup loads, spread across the two HWDGE queues.
            # scalar queue: wq in k-tile chunks (first projection matmul can
            # start after chunk 0), wkv, biases. sync queue: x stream (phase 1
            # loop below), rope tables, then wo/tri (needed only in phase 2).
            wq_sb = pp.tile([128, NDT, CQ], BF16)
            for wc in range(4):
                nc.scalar.dma_start(wq_sb[:, ts(wc, 4), :],
                                    wq.ap().rearrange("(a p) c -> p a c", p=128)
                                    [:, ts(wc, 4), :])
            wkv_sb = pp.tile([128, NDT, 128], BF16)
            nc.scalar.dma_start(wkv_sb[:], wkv.ap().rearrange("(a p) c -> p a c", p=128))
            bq_sb = pp.tile([128, 2], F32)
            nc.scalar.dma_start(bq_sb[:], bqd.ap().rearrange("(t p) -> p t", p=128))
            bkv_sb = pp.tile([128, 1], F32)
            nc.scalar.dma_start(bkv_sb[:], bkvd.ap()[:, None])
            ones_pe = pp.tile([65, 64], F32R, name="ones_pe")
            nc.gpsimd.dma_start(ones_pe[64:65, :], onesfd.ap()[None, :])
            cc_sb = pp.tile([128, S], BF16)
            ss_sb = pp.tile([128, S], BF16)
            wo_sb = pp.tile([128, 2, DIM], BF16)
            if causal:
                tri_sb = pp.tile([128, 4, 2, QT], BF16)
            if not USE_DMA_TRANSPOSE:
                ident = pp.tile([64, 64], BF16)
                make_identity(nc, ident[:])

            # persistent activations
            qT = [pp.tile([128, S], BF16, tag=f"qt{i}", name=f"qt{i}") for i in range(2)]
            kT = pp.tile([128, S], BF16)          # rows 0:64 and 64:128 both = roped k
            v_sb = pp.tile([128, NKT, 65], BF16)  # [kpos, ktile, ch + ones]
            nc.gpsimd.dma_start(v_sb[:, :, 64:65],
                                onesd.ap()[:, None, None].to_broadcast((128, NKT, 1)))
            attn = [[pp.tile([128, QT], BF16, tag=f"attn{c}_{t}", name=f"attn{c}_{t}")
                     for t in range(NQT)] for c in range(2)]

            # ------- Phase 1: QKV projections + fused RoPE + v transpose -------
            with tc.tile_pool(name="xstream", bufs=2) as xp, \
                 tc.tile_pool(name="qkvps", bufs=2, space="PSUM") as qps, \
                 (nullcontext() if USE_DMA_TRANSPOSE else
                  tc.tile_pool(name="vtps", bufs=2, space="PSUM")) as vps, \
                 tc.tile_pool(name="rawp", bufs=2) as rawp, \
                 tc.tile_pool(name="rope", bufs=3) as rp:

                def rope(raw_ap, out_ap, rows, ssl):
                    sw = rp.tile([128, ST], BF16, tag="ropesw")
                    t1 = rp.tile([128, ST], BF16, tag="ropet1")
                    for b in rows:
                        nc.gpsimd.dma_start(sw[b:b + 32, :], raw_ap[b + 32:b + 64, :])
                        nc.gpsimd.dma_start(sw[b + 32:b + 64, :], raw_ap[b:b + 32, :])
                    lo, hi = rows[0], rows[-1] + 64
                    nc.vector.tensor_mul(t1[lo:hi, :], raw_ap[lo:hi, :], cc_sb[lo:hi, ssl])
                    nc.vector.tensor_mul(sw[lo:hi, :], sw[lo:hi, :], ss_sb[lo:hi, ssl])
                    nc.vector.tensor_add(out_ap[lo:hi, ssl], t1[lo:hi, :], sw[lo:hi, :])

                for st in range(NST):
                    ssl = ts(st, ST)
                    xt = xp.tile([128, NDT, ST], BF16, tag="xt")
                    xr = xT.ap().rearrange("(a p) s -> p a s", p=128)
                    for qtr in range(4):
                        nc.sync.dma_start(xt[:, ts(qtr, 4), :], xr[:, ts(qtr, 4), ssl])
                    if st == 0:
                        # rope tables after the first x tile; phase-2-only
                        # tensors at the back of the sync queue
                        nc.sync.dma_start(cc_sb[:], ccd[:])
                        nc.sync.dma_start(ss_sb[:], ssd[:])
                    if st == NST - 1:
                        nc.sync.dma_start(wo_sb[:],
                                          wo.ap().rearrange("(a p) e -> p a e", p=128))
                        if causal:
                            nc.sync.dma_start(tri_sb[:], trid[:])
                    pq0 = qps.tile([128, ST], F32, tag="q0")
                    pq1 = qps.tile([128, ST], F32, tag="q1")
                    pkv = qps.tile([128, ST], F32, tag="kv")
                    for kt in range(NDT):
                        st_flag, sp_flag = kt == 0, kt == NDT - 1
                        nc.tensor.matmul(pq0[:], wq_sb[:, kt, 0:128], xt[:, kt, :],
                                         start=st_flag, stop=sp_flag)
                        nc.tensor.matmul(pq1[:], wq_sb[:, kt, 128:256], xt[:, kt, :],
                                         start=st_flag, stop=sp_flag)
                        nc.tensor.matmul(pkv[:], wkv_sb[:, kt, :], xt[:, kt, :],
                                         start=st_flag, stop=sp_flag)
                    q0_raw = rawp.tile([128, ST], BF16, tag="q0r")
                    q1_raw = rawp.tile([128, ST], BF16, tag="q1r")
                    kv_raw = rawp.tile([128, ST], BF16, tag="kvr")
                    nc.scalar.activation(q0_raw[:], pq0[:],
                                         mybir.ActivationFunctionType.Identity,
                                         bias=bq_sb[:, 0:1])
                    nc.scalar.activation(q1_raw[:], pq1[:],
                                         mybir.ActivationFunctionType.Identity,
                                         bias=bq_sb[:, 1:2])
                    nc.scalar.activation(kv_raw[:], pkv[:],
                                         mybir.ActivationFunctionType.Identity,
                                         bias=bkv_sb[:, 0:1])
                    rope(q0_raw[:], qT[0][:], [0, 64], ssl)
                    rope(q1_raw[:], qT[1][:], [0, 64], ssl)
                    rope(kv_raw[:], kT[:], [64], ssl)
                    nc.gpsimd.dma_start(kT[0:64, ssl], kT[64:128, ssl])
                    for vc in range(ST // 128):
                        j = (st * ST) // 128 + vc
                        if USE_DMA_TRANSPOSE:
                            nc.sync.dma_start(v_sb[:, j, 0:64],
                                              kv_raw[0:64, ts(vc, 128)],
                                              transpose=True)
                        else:
                            vt_ps = vps.tile([128, 64], BF16, tag="vt")
                            nc.tensor.transpose(vt_ps[:], kv_raw[0:64, ts(vc, 128)],
                                                ident[:])
                            nc.vector.tensor_copy(v_sb[:, j, 0:64], vt_ps[:])

            # ------- Phase 2+3 fused: attention with interleaved out-proj -------
            with tc.tile_pool(name="scps", bufs=2, space="PSUM") as scps, \
                 tc.tile_pool(name="avps", bufs=4, space="PSUM") as avps, \
                 tc.tile_pool(name="ptp", bufs=16) as ptp, \
                 tc.tile_pool(name="nrm", bufs=3) as nrm, \
                 tc.tile_pool(name="osb", bufs=8) as osb, \
                 (tc.tile_pool(name="mskp", bufs=4) if use_mask else nullcontext()) as mskp:
                def emit_outproj(pps, t, sl, et, eng="dve"):
                    for ct in range(2):
                        nc.tensor.matmul(pps[:],
                                         attn[ct][t][:, ts(sl, 128)],
                                         wo_sb[:, ct, ts(et, 512)],
                                         start=(ct == 0), stop=(ct == 1))
                    ot = osb.tile([128, QT], BF16, tag="ot")
                    if eng == "act":
                        nc.scalar.activation(ot[:], pps[:],
                                             mybir.ActivationFunctionType.Identity)
                    else:
                        nc.vector.tensor_copy(ot[:], pps[:])
                    nc.sync.dma_start(
                        partial[ts(4 * t + sl, 128), ts(et, 512)], ot[:])

                def finish_norm(t, svs, rcs):
                    # bc matmuls + normalization multiplies for t, emitted
                    # early in t+1's j-loop so the in-order PE queue reaches
                    # them only once their reciprocals are long done
                    for h in range(4):
                        hp, hh = divmod(h, 2)
                        bc = scps.tile([64, QT], F32, tag="sc", name="bc")
                        nc.tensor.matmul(bc[:], ones_pe[64:65, 0:64],
                                         rcs[h][64:65, :],
                                         start=True, stop=True)
                        if DEBUG_DUMP and t == 0:
                            bcs = nrm.tile([64, QT], F32, tag="bcs")
                            nc.vector.tensor_copy(bcs[:], bc[:])
                            nc.sync.dma_start(bcs_d.ap()[:, h], bcs[:])
                        if hh == 0:
                            nc.vector.tensor_mul(attn[hp][t][0:64, :],
                                                 svs[h][0:64, :], bc[:])
                        else:
                            tb = nrm.tile([64, QT], BF16, tag="tb")
                            nc.vector.tensor_mul(tb[:], svs[h][0:64, :], bc[:])
                            nc.gpsimd.dma_start(attn[hp][t][64:128, :], tb[:])

                pending = []   # out-proj work of the previous t, interleaved
                norm_prev = None  # (t-1, aps, reciprocal rows)
                for t in range(NQT):
                    tsl = ts(t, QT)
                    n_k = 4 * (t + 1) if causal else NKT
                    # slot rotation: aps(t) land on the slots of op(t-2)#12..15
                    # (long done), and op(t-1) tiles land on aps(t-1)'s slots,
                    # which free exactly when t-1's norm multiplies complete
                    aps = [avps.tile([65, QT], F32, tag="av", name=f"av{t}_{h}")
                           for h in range(4)]
                    pending = [(avps.tile([128, QT], F32, tag="av", name="op"),
                                *w) for w in pending]
                    for j in range(n_k):
                        jsl = ts(j, 128)
                        if j == 1 and norm_prev is not None:
                            finish_norm(*norm_prev)
                            norm_prev = None
                        if j >= 2:
                            for _ in range(2):
                                if pending:
                                    emit_outproj(*pending.pop(0))
                        if use_mask:
                            mt = mskp.tile([128, QT], BF16, tag="mt")
                            nc.sync.dma_start(mt[:], maskt[jsl, tsl])
                        scs = []
                        for hp in range(2):
                            sc = scps.tile([128, 2, QT], F32, tag="sc", name="sc")
                            nc.tensor.matmul(sc[:, 0, :], kT[0:64, jsl],
                                             qT[hp][0:64, tsl],
                                             start=True, stop=True)
                            scs.append(sc)
                        for hp in range(2):
                            nc.tensor.matmul(scs[hp][:, 1, :], kT[64:128, jsl],
                                             qT[hp][64:128, tsl],
                                             start=True, stop=True)
                        pts_dbg = []
                        for hp in range(2):
                            pt = ptp.tile([128, 2, QT], BF16, tag="pt")
                            pts_dbg.append(pt)
                            nc.scalar.activation(pt[:], scs[hp][:],
                                                 mybir.ActivationFunctionType.Exp,
                                                 scale=0.125)
                            if causal and j >= 4 * t:
                                nc.vector.tensor_mul(pt[:], pt[:],
                                                     tri_sb[:, j - 4 * t])
                            if use_mask:
                                for c in range(2):
                                    nc.vector.tensor_mul(pt[:, c, :], pt[:, c, :],
                                                         mt[:])
                            for c in range(2):
                                nc.tensor.matmul(aps[2 * hp + c][:], v_sb[:, j, :],
                                                 pt[:, c, :],
                                                 start=(j == 0), stop=(j == n_k - 1))
                        if DEBUG_DUMP and t == 0 and j == 0:
                            for hpd in range(2):
                                nc.sync.dma_start(pt_d.ap()[:, hpd], pts_dbg[hpd][:])
                    # drain any out-proj leftovers of t-1 (short j-loops)
                    while pending:
                        emit_outproj(*pending.pop(0))
                    # reciprocals of this t's softmax sums (DVE only; the
                    # dependent bc matmuls and multiplies are deferred)
                    rcs, svs = [], []
                    for h in range(4):
                        sv = nrm.tile([65, QT], F32, tag="sv", bufs=8)
                        nc.scalar.activation(sv[:], aps[h][:],
                                             mybir.ActivationFunctionType.Identity)
                        if DEBUG_DUMP and t == 0:
                            nc.sync.dma_start(aps_d.ap()[:, h], sv[:])
                        svs.append(sv)
                        rc = nrm.tile([65, QT], F32, tag="rc", bufs=8)
                        nc.vector.reciprocal_approx_fast(rc[:], sv[:])
                        if DEBUG_DUMP and t == 0:
                            nc.sync.dma_start(rc_d.ap()[:, h], rc[:])
                        rcs.append(rc)
                    rcbs = []
                    for h in range(4):
                        rcb = nrm.tile([65, QT], F32R, tag="rcb", bufs=8)
                        nc.scalar.activation(rcb[64:65, :], rcs[h][64:65, :],
                                             mybir.ActivationFunctionType.Copy)
                        rcbs.append(rcb)
                    norm_prev = (t, svs, rcbs)
                    pending = [(t, sl, et, "act" if (4 * sl + et) % 2 else "dve")
                               for sl in range(4) for et in range(4)]
                if DEBUG_DUMP:
                    nc.sync.dma_start(kT_d[:], kT[:])
                    nc.sync.dma_start(qT_d[:], qT[0][:])
                    nc.sync.dma_start(v_d.ap().rearrange("p (a c) -> p a c", a=NKT), v_sb[:])
                    nc.sync.dma_start(attn_d[:], attn[0][0][:])
                # tail: the last t's norm + out-projection
                finish_norm(*norm_prev)
                for w in pending:
                    emit_outproj(avps.tile([128, QT], F32, tag="av", name="op"), *w)

    nc.compile()
    return nc


_CACHE = {}
TRACE = False
LAST_EXEC_NS = None
LAST_RES = None


def _get(causal, use_mask):
    key = (causal, use_mask)
    if key not in _CACHE:
        _CACHE[key] = _build(causal, use_mask)
    return _CACHE[key]


def _perm_eo(w):
    # de-interleave channel pairs per 64-col head block: [evens, odds]
    cols = np.concatenate([np.arange(0, 64, 2), np.arange(1, 64, 2)])
    return w[..., cols]


def _bf(a):
    return np.ascontiguousarray(np.asarray(a, dtype=np.float32).astype(ml_dtypes.bfloat16))


def kernel(**inputs):
    x = np.asarray(inputs["x"], dtype=np.float32)
    fc = np.asarray(inputs["freqs_cos"], dtype=np.float32)
    fs = np.asarray(inputs["freqs_sin"], dtype=np.float32)
    mask = np.asarray(inputs["mask"])
    Wq = np.asarray(inputs["Wq"], dtype=np.float32)
    bq = np.asarray(inputs["bq"], dtype=np.float32)
    Wk = np.asarray(inputs["Wk"], dtype=np.float32)
    bk = np.asarray(inputs["bk"], dtype=np.float32)
    Wv = np.asarray(inputs["Wv"], dtype=np.float32)
    bv = np.asarray(inputs["bv"], dtype=np.float32)
    Wo = np.asarray(inputs["Wo"], dtype=np.float32)
    bo = np.asarray(inputs["bo"], dtype=np.float32)

    m2 = mask.reshape(S, S)
    if (m2 == 1).all():
        causal, use_mask = False, False
    elif np.array_equal(m2 != 0, np.tril(np.ones((S, S), dtype=bool))):
        causal, use_mask = True, False
    else:
        causal, use_mask = False, True
    nc = _get(causal, use_mask)

    xT = _bf(x[0].T)
    cosT = np.asarray(fc.T, dtype=np.float32)  # (32, S)
    sinT = np.asarray(fs.T, dtype=np.float32)
    cc = _bf(np.tile(cosT, (4, 1)))
    ssgn = _bf(np.concatenate([-sinT, sinT, -sinT, sinT], axis=0))
    kl = np.arange(128)[:, None]
    qq = np.arange(QT)[None, :]
    tri = np.stack([(qq >= 128 * v + kl) for v in range(4)]).astype(np.float32)
    tri2 = _bf(np.broadcast_to(tri[None, :, :, :], (2, 4, 128, QT))
               .transpose(2, 1, 0, 3))  # [128, 4, 2, QT]

    Wq_h = Wq.reshape(DIM, H, D)
    bq_h = bq.reshape(H, D)
    Wk_h = Wk.reshape(DIM, KVH, D)
    bk_h = bk.reshape(KVH, D)

    in_maps = []
    for c in range(NCORES):
        hs = slice(HQ * c, HQ * (c + 1))
        wq_c = _perm_eo(Wq_h[:, hs, :]).reshape(DIM, CQ)
        bq_c = _perm_eo(bq_h[hs, :]).reshape(CQ)
        wk_c = _perm_eo(Wk_h[:, c, :])
        bk_c = _perm_eo(bk_h[c, :])
        wv_c = Wv[:, 64 * c:64 * (c + 1)]
        bv_c = bv[64 * c:64 * (c + 1)]
        wkv_c = np.concatenate([wv_c, wk_c], axis=1)
        bkv_c = np.concatenate([bv_c, bk_c])
        wo_c = Wo[CQ * c:CQ * (c + 1), :]
        im = {
            "xT": xT, "wq": _bf(wq_c), "wkv": _bf(wkv_c),
            "wo": _bf(wo_c), "bq": np.ascontiguousarray(bq_c),
            "bkv": np.ascontiguousarray(bkv_c), "cc": cc,
            "ssgn": ssgn,
            "ones": np.ones(128, dtype=ml_dtypes.bfloat16),
            "onesf": np.ones(64, dtype=np.float32),
        }
        if causal:
            im["trimask"] = tri2
        if use_mask:
            im["maskt"] = _bf(m2.T)
        in_maps.append(im)

    global LAST_EXEC_NS, LAST_RES
    res = run_bass_kernel_spmd(nc, in_maps, core_ids=list(range(NCORES)), trace=TRACE)
    LAST_EXEC_NS = res.exec_time_ns
    LAST_RES = res
    out = np.zeros((S, DIM), dtype=np.float32)
    for rr in res.results:
        out += np.asarray(rr["partial"], dtype=np.float32)
    out += bo
    return out.reshape(1, S, DIM)
